# revision 5
# baseline (speedup 1.0000x reference)
"""Batched quantize->matmul->dequantize kernel for 8 Trainium2 NeuronCores.

Problem: input0 [16,1024,1024] f32, input1 [16,1024,1024] f32.
  qa = clip(round(input0*10), -128, 127); qb likewise
  out = (qa @ qb) / 10            # batched, f32

Strategy: shard the batch dim across 8 cores (2 batches/core); each core runs
an identical Bass/Tile kernel with no communication.

Quantization: one multiply-by-10 with int8 output -- the hardware f32->int8
conversion is round-to-nearest-even with saturation, which is exactly
jnp.clip(jnp.round(x*10), -128, 127) (verified on device incl. the
double-rounding and saturation edge cases). The int8 is cast to bf16 for
the PE: ints <= 128 are exact in bf16, products are exact in the PE's
multiply, and the f32 PSUM accumulation of integer partial sums < 2^24 is
exact, so the pre-dequant matmul matches the reference bit-for-bit.

Outputs are written as bf16 (dequant x0.1 fused into the PSUM->SBUF
eviction) and widened to f32 on the host: |out| <= ~2e3 here, so bf16
rounding is <= 2^-9 of the element magnitude -- two orders below the 2e-2
acceptance threshold -- and it halves the output DMA bytes.

Schedule (from the measured trace of the previous version): the wall is the
serial PE matmul stream plus fixed ends -- first data at ~9 us and a
semaphore-teardown epilogue that scales with instruction count. Changes vs
that version, each from a measured loss:
 - the warmup chain (HAM clock-gate release) shrinks 46 -> 12 MMs of
   [128,128]: the PE queue is FIFO, so the old chain blocked the first
   real matmul until ~19.5 us while data was ready at ~13 us.
 - matmuls stay 2x512-wide per (m,k): walrus's s3d3_mm check caps the
   per-instruction free dim at 512 (one PSUM bank).
 - k-outer over (4,3,1) m-tile groups per batch; evictions fire per tile
   as its stop-MM retires, overlapping the next group's matmuls, so PSUM
   buffer reuse costs no bubble.
 - input DMAs are 1 MiB each (first k-tile of batch 0 split 2x512 KiB so
   the first real MM starts early); outputs coalesce into 6 DMAs. All on
   the sync queue: program order = transfer order, inputs before outputs.
 - LDWEIGHTS bursts in batch-0's first group keep the HAM activity monitor
   from clock-throttling across ingest-paced PE stalls (PSUM's 8 banks cap
   executable work at ~3.4 us per ~5 us k-pair arrival there).
"""

import sys

if "/opt/trn_rl_repo" not in sys.path:
    sys.path.insert(0, "/opt/trn_rl_repo")

import numpy as np

import concourse.bass as bass
import concourse.mybir as mybir
import concourse.tile as tile
from concourse import bacc
from concourse.bass_utils import run_bass_kernel_spmd

N_CORES = 8
B, M, K, N = 16, 1024, 1024, 1024
BPC = B // N_CORES  # batches per core
P = 128
KT = K // P  # k tiles per batch
MT = M // P  # m tiles per batch
KP = KT // 2  # k-tile pairs (1 MiB DMA granularity)

DSCALE = 10.0
WSCALE = 10.0
OSCALE = 10.0

f32 = mybir.dt.float32
bf16 = mybir.dt.bfloat16
i8 = mybir.dt.int8


def _build_kernel(nc: bass.Bass):
    # A arrives pre-arranged [BPC, K, M]; B natural [BPC, K, N].
    a_dram = nc.dram_tensor("input0_t", [BPC, K, M], f32, kind="ExternalInput").ap()
    b_dram = nc.dram_tensor("input1", [BPC, K, N], f32, kind="ExternalInput").ap()
    c_dram = nc.dram_tensor("output", [BPC, M, N], bf16, kind="ExternalOutput").ap()

    with tile.TileContext(nc) as tc:
        with (
            tc.tile_pool(name="warm", bufs=1) as warm_pool,
            tc.tile_pool(name="a_f32", bufs=3) as a_pool,
            tc.tile_pool(name="b_f32", bufs=3) as b_pool,
            tc.tile_pool(name="a_i8", bufs=2) as ai_pool,
            tc.tile_pool(name="b_i8", bufs=2) as bi_pool,
            tc.tile_pool(name="qa", bufs=BPC * KP) as qa_pool,
            tc.tile_pool(name="qb", bufs=BPC * KP) as qb_pool,
            tc.tile_pool(name="psum", bufs=4, space="PSUM") as psum_pool,
            tc.tile_pool(name="c_bf16", bufs=3) as c_pool,
        ):
            # PE warmup: a short [128,128] chain releases the HAM clock
            # gate without blocking the FIFO PE queue past first-data.
            wsrc = warm_pool.tile([P, 512], bf16)
            nc.gpsimd.memset(wsrc[:], 0.0)
            wps = psum_pool.tile([P, N], f32, tag="ps", name="wps")
            for _ in range(12):
                nc.tensor.matmul(wps[:, :P], wsrc[:, :P], wsrc[:, :P],
                                 start=True, stop=True)

            # ---- input ingest + quantization ----------------------------
            # Program order on the sync queue = transfer order on its
            # single HWDGE ring: every input byte moves before any output.
            qa = [[] for _ in range(BPC)]
            qb = [[] for _ in range(BPC)]

            def load_pair(pool, ipool, qpool, dram, b, kp, scale, split, cast):
                """DMA one k-pair [128, 2048] f32 and quantize to bf16."""
                t_ = pool.tile([P, 2048], f32, tag="st", name=f"{pool.name}{b}_{kp}")
                r0 = 2 * kp * P
                if split:
                    for t in range(2):
                        nc.sync.dma_start(
                            out=t_[:, t * 1024 : (t + 1) * 1024],
                            in_=dram[b, r0 + t * P : r0 + (t + 1) * P, :],
                        )
                else:
                    nc.sync.dma_start(
                        out=t_[:].rearrange("p (t m) -> p t m", t=2),
                        in_=dram[b, r0 : r0 + 2 * P, :].rearrange(
                            "(t p) m -> p t m", p=P
                        ),
                    )
                it = ipool.tile([P, 2048], i8, tag="qi", name=f"{ipool.name}{b}_{kp}")
                qt = qpool.tile([P, 2048], bf16, tag="qt", name=f"{qpool.name}{b}_{kp}")
                if split:
                    # per-k-tile quant: each DVE op fires the moment its
                    # half of the DMA lands -> earliest possible first MM
                    for t in range(2):
                        sl = slice(t * 1024, (t + 1) * 1024)
                        nc.vector.tensor_scalar_mul(it[:, sl], t_[:, sl], scale)
                        nc.vector.tensor_copy(out=qt[:, sl], in_=it[:, sl])
                else:
                    nc.vector.tensor_scalar_mul(it[:], t_[:], scale)
                    if cast == "vector":
                        nc.vector.tensor_copy(out=qt[:], in_=it[:])
                    else:
                        nc.scalar.copy(qt[:], it[:])
                return qt

            for b in range(BPC):
                for kp in range(KP):
                    split = b == 0 and kp == 0
                    qa[b].append(
                        load_pair(a_pool, ai_pool, qa_pool, a_dram, b, kp,
                                  DSCALE, split, "vector")
                    )
                    qb[b].append(
                        load_pair(b_pool, bi_pool, qb_pool, b_dram, b, kp,
                                  WSCALE, split, "vector" if split else "scalar")
                    )

            # ---- matmul + eviction + output -----------------------------
            # k-outer so the PE consumes each k-pair as it streams in.
            # Groups of (4,3,1) m-tiles: 4 PSUM bufs of [128,1024] (8 banks)
            # cycle across groups; each tile's eviction fires as its stop-MM
            # retires, overlapping the next group's matmuls. The last group
            # is 1 tile so a single eviction gates the final output DMA.
            for b in range(BPC):
                for m0, gsz in ((0, 4), (4, 3), (7, 1)):
                    ps = [
                        psum_pool.tile([P, N], f32, tag="ps", name=f"ps{b}_{m0}_{i}")
                        for i in range(gsz)
                    ]
                    for k in range(KT):
                        kp, t = divmod(k, 2)
                        for mi in range(gsz):
                            m = m0 + mi
                            lhsT = qa[b][kp][:, t * 1024 + m * P : t * 1024 + (m + 1) * P]
                            for nh in range(2):
                                # moving operand / PSUM write cap is 512
                                # (one PSUM bank) per matmul instruction
                                nc.tensor.matmul(
                                    ps[mi][:, nh * 512 : (nh + 1) * 512],
                                    lhsT,
                                    qb[b][kp][
                                        :, t * 1024 + nh * 512 : t * 1024 + (nh + 1) * 512
                                    ],
                                    start=(k == 0),
                                    stop=(k == KT - 1),
                                )
                        if b == 0 and m0 == 0 and k in (1, 3, 5):
                            # batch 0's first sweep is ingest-paced; these
                            # weight loads keep the PE activity monitor from
                            # clock-throttling during the arrival stalls
                            for _ in range(12):
                                nc.tensor.ldweights(wsrc[:, :P])
                    ct = c_pool.tile([P, gsz * N], bf16, tag="ct", name=f"ct{b}_{m0}")
                    ct3 = ct[:].rearrange("p (g n) -> p g n", g=gsz)
                    for h in range(gsz):
                        # dequant + bf16 cast fused into the PSUM eviction
                        nc.scalar.activation(
                            ct3[:, h, :],
                            ps[h][:],
                            mybir.ActivationFunctionType.Copy,
                            scale=1.0 / OSCALE,
                        )
                    nc.sync.dma_start(
                        out=c_dram[b, m0 * P : (m0 + gsz) * P, :].rearrange(
                            "(g p) n -> p g n", p=P
                        ),
                        in_=ct3,
                    )


_NC_CACHE = None


def _get_nc():
    global _NC_CACHE
    if _NC_CACHE is None:
        nc = bacc.Bacc("TRN2", target_bir_lowering=False, debug=False,
                       num_devices=N_CORES)
        _build_kernel(nc)
        nc.compile()
        _NC_CACHE = nc
    return _NC_CACHE


def _make_in_maps(input0: np.ndarray, input1: np.ndarray):
    in_maps = []
    for c in range(N_CORES):
        sl = slice(c * BPC, (c + 1) * BPC)
        a_t = np.ascontiguousarray(input0[sl].transpose(0, 2, 1))
        in_maps.append(
            {"input0_t": a_t, "input1": np.ascontiguousarray(input1[sl])}
        )
    return in_maps


def kernel(input0, input1, **run_kwargs):
    input0 = np.asarray(input0, dtype=np.float32)
    input1 = np.asarray(input1, dtype=np.float32)
    assert input0.shape == (B, M, K) and input1.shape == (B, K, N)

    nc = _get_nc()
    in_maps = _make_in_maps(input0, input1)
    res = None
    for attempt in range(3):
        try:
            res = run_bass_kernel_spmd(
                nc, in_maps, core_ids=list(range(N_CORES)), **run_kwargs,
            )
            break
        except Exception:
            if attempt == 2:
                raise
    assert res is not None
    out = np.concatenate(
        [np.asarray(res.results[c]["output"]) for c in range(N_CORES)], axis=0
    ).astype(np.float32)
    if run_kwargs:
        return out, res
    return out


if __name__ == "__main__":
    a = np.random.randn(B, M, K).astype(np.float32)
    bm = np.random.randn(B, K, N).astype(np.float32)
    out = kernel(a, bm)
    print("out", out.shape, out.dtype)


# revision 8
# speedup vs baseline: 1.0562x; 1.0562x over previous
"""Batched quantize->matmul->dequantize kernel for 8 Trainium2 NeuronCores.

Problem: input0 [16,1024,1024] f32, input1 [16,1024,1024] f32.
  qa = clip(round(input0*10), -128, 127); qb likewise
  out = (qa @ qb) / 10            # batched, f32

Strategy: shard the batch dim across 8 cores (2 batches/core); each core runs
an identical Bass/Tile kernel with no communication.

Quantization: one multiply-by-10 with int8 output -- the hardware f32->int8
conversion is round-to-nearest-even with saturation, which is exactly
jnp.clip(jnp.round(x*10), -128, 127) (verified on device incl. the
double-rounding and saturation edge cases). The int8 is cast to bf16 for
the PE: ints <= 128 are exact in bf16, products are exact in the PE's
multiply, and the f32 PSUM accumulation of integer partial sums < 2^24 is
exact, so the pre-dequant matmul matches the reference bit-for-bit.

Outputs are written as bf16 (dequant x0.1 fused into the PSUM->SBUF
eviction) and widened to f32 on the host: |out| <= ~2e3 here, so bf16
rounding is <= 2^-9 of the element magnitude -- two orders below the 2e-2
acceptance threshold -- and it halves the output DMA bytes.

Schedule (from the measured trace of the previous version): the wall is the
serial PE matmul stream plus fixed ends -- first data at ~9 us and a
semaphore-teardown epilogue that scales with instruction count. Changes vs
that version, each from a measured loss:
 - the warmup chain (HAM clock-gate release) shrinks 46 -> 12 MMs of
   [128,128]: the PE queue is FIFO, so the old chain blocked the first
   real matmul until ~19.5 us while data was ready at ~13 us.
 - matmuls stay 2x512-wide per (m,k): walrus's s3d3_mm check caps the
   per-instruction free dim at 512 (one PSUM bank).
 - k-outer over (4,3,1) m-tile groups per batch; evictions fire per tile
   as its stop-MM retires, overlapping the next group's matmuls, so PSUM
   buffer reuse costs no bubble.
 - input DMAs are 1 MiB each (first k-tile of batch 0 split 2x512 KiB so
   the first real MM starts early); outputs coalesce into 6 DMAs. All on
   the sync queue: program order = transfer order, inputs before outputs.
 - LDWEIGHTS bursts in batch-0's first group keep the HAM activity monitor
   from clock-throttling across ingest-paced PE stalls (PSUM's 8 banks cap
   executable work at ~3.4 us per ~5 us k-pair arrival there).
"""

import sys

if "/opt/trn_rl_repo" not in sys.path:
    sys.path.insert(0, "/opt/trn_rl_repo")

import numpy as np

import concourse.bass as bass
import concourse.mybir as mybir
import concourse.tile as tile
from concourse import bacc
from concourse.bass_utils import run_bass_kernel_spmd

N_CORES = 8
B, M, K, N = 16, 1024, 1024, 1024
BPC = B // N_CORES  # batches per core
P = 128
KT = K // P  # k tiles per batch
MT = M // P  # m tiles per batch
KP = KT // 2  # k-tile pairs (1 MiB DMA granularity)

DSCALE = 10.0
WSCALE = 10.0
OSCALE = 10.0

f32 = mybir.dt.float32
bf16 = mybir.dt.bfloat16
i8 = mybir.dt.int8


def _build_kernel(nc: bass.Bass):
    # A arrives pre-arranged [BPC, K, M]; B natural [BPC, K, N].
    a_dram = nc.dram_tensor("input0_t", [BPC, K, M], f32, kind="ExternalInput").ap()
    b_dram = nc.dram_tensor("input1", [BPC, K, N], f32, kind="ExternalInput").ap()
    c_dram = nc.dram_tensor("output", [BPC, M, N], bf16, kind="ExternalOutput").ap()

    with tile.TileContext(nc) as tc:
        with (
            tc.tile_pool(name="warm", bufs=1) as warm_pool,
            tc.tile_pool(name="a_f32", bufs=3) as a_pool,
            tc.tile_pool(name="b_f32", bufs=3) as b_pool,
            tc.tile_pool(name="a_i8", bufs=2) as ai_pool,
            tc.tile_pool(name="b_i8", bufs=2) as bi_pool,
            tc.tile_pool(name="qa", bufs=BPC * KP) as qa_pool,
            tc.tile_pool(name="qb", bufs=BPC * KP) as qb_pool,
            tc.tile_pool(name="psum", bufs=4, space="PSUM") as psum_pool,
            tc.tile_pool(name="c_bf16", bufs=3) as c_pool,
        ):
            # PE warmup: the HAM clock gate needs ~4 us of sustained PE
            # activity before it releases the full 2.4 GHz clock -- and it
            # gates the whole NeuronCore, so an under-sized warmup also
            # halves the DVE quant rate. 12 [128,512] MMs give ~5 us of
            # activity and drain right as the first k-tile's quant lands,
            # so they never block a data-ready real matmul.
            wsrc = warm_pool.tile([P, 512], bf16)
            nc.gpsimd.memset(wsrc[:], 0.0)
            wps = psum_pool.tile([P, N], f32, tag="ps", name="wps")
            for _ in range(12):
                nc.tensor.matmul(wps[:, :512], wsrc[:, :P], wsrc[:],
                                 start=True, stop=True)

            # ---- input ingest + quantization ----------------------------
            # Program order on the sync queue = transfer order on its
            # single HWDGE ring: every input byte moves before any output.
            qa = [[] for _ in range(BPC)]
            qb = [[] for _ in range(BPC)]

            def load_pair(pool, ipool, qpool, dram, b, kp, scale, split, cast):
                """DMA one k-pair [128, 2048] f32 and quantize to bf16."""
                t_ = pool.tile([P, 2048], f32, tag="st", name=f"{pool.name}{b}_{kp}")
                r0 = 2 * kp * P
                # plain [128,1024] transfers: a fused [128,2,1024] 3D AP
                # measured ~12% slower (362 vs 413 GB/s issue pace)
                for t in range(2):
                    nc.sync.dma_start(
                        out=t_[:, t * 1024 : (t + 1) * 1024],
                        in_=dram[b, r0 + t * P : r0 + (t + 1) * P, :],
                    )
                it = ipool.tile([P, 2048], i8, tag="qi", name=f"{ipool.name}{b}_{kp}")
                qt = qpool.tile([P, 2048], bf16, tag="qt", name=f"{qpool.name}{b}_{kp}")
                if split:
                    # per-k-tile quant: each DVE op fires the moment its
                    # half of the DMA lands -> earliest possible first MM
                    for t in range(2):
                        sl = slice(t * 1024, (t + 1) * 1024)
                        nc.vector.tensor_scalar_mul(it[:, sl], t_[:, sl], scale)
                        nc.vector.tensor_copy(out=qt[:, sl], in_=it[:, sl])
                else:
                    nc.vector.tensor_scalar_mul(it[:], t_[:], scale)
                    if cast == "vector":
                        nc.vector.tensor_copy(out=qt[:], in_=it[:])
                    else:
                        nc.scalar.copy(qt[:], it[:])
                return qt

            for b in range(BPC):
                for kp in range(KP):
                    split = b == 0 and kp == 0
                    qa[b].append(
                        load_pair(a_pool, ai_pool, qa_pool, a_dram, b, kp,
                                  DSCALE, split, "vector")
                    )
                    qb[b].append(
                        load_pair(b_pool, bi_pool, qb_pool, b_dram, b, kp,
                                  WSCALE, split, "vector" if split else "scalar")
                    )

            # ---- matmul + eviction + output -----------------------------
            # k-outer so the PE consumes each k-pair as it streams in.
            # Groups of (4,3,1) m-tiles: 4 PSUM bufs of [128,1024] (8 banks)
            # cycle across groups; each tile's eviction fires as its stop-MM
            # retires, overlapping the next group's matmuls. The last group
            # is 1 tile so a single eviction gates the final output DMA.
            for b in range(BPC):
                for m0, gsz in ((0, 4), (4, 3), (7, 1)):
                    ps = [
                        psum_pool.tile([P, N], f32, tag="ps", name=f"ps{b}_{m0}_{i}")
                        for i in range(gsz)
                    ]
                    for k in range(KT):
                        kp, t = divmod(k, 2)
                        for mi in range(gsz):
                            m = m0 + mi
                            lhsT = qa[b][kp][:, t * 1024 + m * P : t * 1024 + (m + 1) * P]
                            for nh in range(2):
                                # moving operand / PSUM write cap is 512
                                # (one PSUM bank) per matmul instruction
                                nc.tensor.matmul(
                                    ps[mi][:, nh * 512 : (nh + 1) * 512],
                                    lhsT,
                                    qb[b][kp][
                                        :, t * 1024 + nh * 512 : t * 1024 + (nh + 1) * 512
                                    ],
                                    start=(k == 0),
                                    stop=(k == KT - 1),
                                )
                        if b == 0 and m0 == 0 and k in (1, 3, 5):
                            # batch 0's first sweep is ingest-paced; these
                            # weight loads keep the PE activity monitor from
                            # clock-throttling during the arrival stalls
                            for _ in range(12):
                                nc.tensor.ldweights(wsrc[:, :P])
                    ct = c_pool.tile([P, gsz * N], bf16, tag="ct", name=f"ct{b}_{m0}")
                    ct3 = ct[:].rearrange("p (g n) -> p g n", g=gsz)
                    final = b == BPC - 1 and m0 == MT - 1
                    for h in range(gsz):
                        # dequant + bf16 cast fused into the PSUM eviction;
                        # the very last tile evicts in halves so its output
                        # DMA starts half an eviction earlier
                        nhalves = 2 if final else 1
                        for q in range(nhalves):
                            sl = slice(q * N // nhalves, (q + 1) * N // nhalves)
                            nc.scalar.activation(
                                ct3[:, h, sl],
                                ps[h][:, sl],
                                mybir.ActivationFunctionType.Copy,
                                scale=1.0 / OSCALE,
                            )
                            if final:
                                nc.sync.dma_start(
                                    out=c_dram[b, m0 * P : (m0 + 1) * P, sl],
                                    in_=ct3[:, 0, sl],
                                )
                    if not final:
                        nc.sync.dma_start(
                            out=c_dram[b, m0 * P : (m0 + gsz) * P, :].rearrange(
                                "(g p) n -> p g n", p=P
                            ),
                            in_=ct3,
                        )

            # Tail warmth: the semaphore-teardown epilogue (~250 sems, all
            # engines) runs 2x slower at the throttled clock, and the PE
            # goes idle right before it. A short dummy chain sized to the
            # eviction+output tail keeps the clock at 8/8 into the teardown
            # without delaying the final all-engine drain barrier.
            wtl = psum_pool.tile([P, N], f32, tag="ps", name="wtail")
            for _ in range(12):
                nc.tensor.matmul(wtl[:, :512], wsrc[:, :P], wsrc[:],
                                 start=True, stop=True)


_NC_CACHE = None


def _get_nc():
    global _NC_CACHE
    if _NC_CACHE is None:
        nc = bacc.Bacc("TRN2", target_bir_lowering=False, debug=False,
                       num_devices=N_CORES)
        _build_kernel(nc)
        nc.compile()
        _NC_CACHE = nc
    return _NC_CACHE


def _make_in_maps(input0: np.ndarray, input1: np.ndarray):
    in_maps = []
    for c in range(N_CORES):
        sl = slice(c * BPC, (c + 1) * BPC)
        a_t = np.ascontiguousarray(input0[sl].transpose(0, 2, 1))
        in_maps.append(
            {"input0_t": a_t, "input1": np.ascontiguousarray(input1[sl])}
        )
    return in_maps


def kernel(input0, input1, **run_kwargs):
    input0 = np.asarray(input0, dtype=np.float32)
    input1 = np.asarray(input1, dtype=np.float32)
    assert input0.shape == (B, M, K) and input1.shape == (B, K, N)

    nc = _get_nc()
    in_maps = _make_in_maps(input0, input1)
    res = None
    for attempt in range(3):
        try:
            res = run_bass_kernel_spmd(
                nc, in_maps, core_ids=list(range(N_CORES)), **run_kwargs,
            )
            break
        except Exception:
            if attempt == 2:
                raise
    assert res is not None
    out = np.concatenate(
        [np.asarray(res.results[c]["output"]) for c in range(N_CORES)], axis=0
    ).astype(np.float32)
    if run_kwargs:
        return out, res
    return out


if __name__ == "__main__":
    a = np.random.randn(B, M, K).astype(np.float32)
    bm = np.random.randn(B, K, N).astype(np.float32)
    out = kernel(a, bm)
    print("out", out.shape, out.dtype)
